# revision 21
# baseline (speedup 1.0000x reference)
"""Trainium2 Bass kernel for nn_Adapter2 (dense_cnn), v3.

Strategy (8 cores, data-parallel over clips, zero collectives), per core:
  xT [768, 197*32] b-major token order (b, l, t) per h-row tile, bf16 + fp8.

  v3 over v2 (206us -> target ~150us). Trace showed DVE 77% busy (160us,
  dominated by ~215ns fixed overhead x 378 stt/ts ops at 1x rate), tensor
  147us (37us cold-throttle), scalar 108us, gpsimd 94us. Changes:
    - Full-width persistent z buffers (zf01/zf2/zg0/zg1) - no pool churn.
    - Spconv taps batched per tile-GROUP over the padded grids with a
      padded accumulator (taps 0..7 contiguous full-row spans; tap 8
      writes the strided z view). Tap op count 252 -> ~66.
    - conv_t-B chain accumulates bf16 on GpSimd (group-batched); the
      unpack DMAs cast bf16->fp8 in-flight (SWDGE cast).
    - conv_t-A / U-diff tap pairs merged across b (1 op each, not 2).
    - PE warm-up: junk matmuls during the initial x DMA keep the HAM
      clock gate open until the first real matmul.
    - One batched store DMA per tile ([128,6,w] osb), one xt load DMA
      per group; eviction bias moved to a 4th constant aux z-row so
      evictions are pure-scale Copy activations.
"""
import sys

if "/opt/trn_rl_repo" not in sys.path:
    sys.path.insert(0, "/opt/trn_rl_repo")

import numpy as np
import ml_dtypes

import concourse.bass as bass
import concourse.mybir as mybir
from concourse.tile import TileContext
from concourse import bass_utils, bacc

F32 = mybir.dt.float32
BF16 = mybir.dt.bfloat16
F8 = mybir.dt.float8e4
AF = mybir.ActivationFunctionType
OP = mybir.AluOpType
PM = mybir.MatmulPerfMode.DoubleRow

C = 768
CA = 192
L = 197
T = 16
NCORES = 8
NL = 32                      # N-columns per core (2 clips x 16 frames)
NCOLS = L * NL               # 6304
HGRID = 14
PADW = 16
ROWE = PADW * T              # 256 elements per padded (h-)row per b
SW = 32.0                    # fp8 weight scale
SZ = 32.0                    # fp8 z scale (= SW, forced by diff path)
G = SW * SZ                  # net fp8-psum scale (gelu w2 pre-scaled by G)

# token tiles: tile 0 = l 0..14 (CLS + h-row 0), tiles 1..13 = h-rows 1..13
TILES = [(0, 15)] + [(1 + 14 * k, 14) for k in range(1, 14)]
TILE_C0 = [0]
for _, _nl in TILES:
    TILE_C0.append(TILE_C0[-1] + _nl * NL)

# tap groups (z/spconv granularity) and load groups (DMA granularity)
TAPG = [[0], [1, 2, 3], [4, 5, 6], [7, 8, 9], [10, 11], [12, 13]]
LOADG = [[0, 1], [2, 3], [4, 5], [6, 7], [8, 9], [10, 11], [12, 13]]
LOADW = [TILE_C0[g[-1] + 1] - TILE_C0[g[0]] for g in LOADG]
MAXLW = max(LOADW)           # 928
# after emit_A(trigger tile), start the DMA for LOADG index value
LOAD_TRIGGER = {1: 3, 3: 4, 5: 5, 7: 6}

NSCAL = 38
TAPS = [(dh, dw) for dh in (-1, 0, 1) for dw in (-1, 0, 1)]

PPADU_COLS = 2 * 16 * PADW * T   # (b, h, w, t) for off ca 64:192 (128p)
PPADP_COLS = 16 * PADW * T       # packed (h, w, t) for off ca 0:64

N_WARM = 32                  # junk matmuls to open the PE clock gate


def _dup(v):
    return np.concatenate([v, v])


def _pack_scalars(conv_w, conv_b, fc1_b, mlp_in_b, off_fc1_b, off_conv_w):
    s = np.zeros((128, NSCAL), np.float32)
    w0, w1, w2 = conv_w[:, 0, 0], conv_w[:, 0, 1], conv_w[:, 0, 2]
    wsum_b = (w0 + w1 + w2) * fc1_b + conv_b
    wsp = off_conv_w[:, 0, 0, :, :]          # (CA, 3, 3)
    # conv_t chunk A (fc1 ca 0:128)
    s[:, 0] = w0[:128]; s[:, 1] = w1[:128]; s[:, 2] = w2[:128]
    s[:, 3] = SZ * wsum_b[:128]
    # conv_t chunk B packed (fc1 ca 128:192 on both halves)
    s[:, 4] = _dup(w0[128:]); s[:, 5] = _dup(w1[128:]); s[:, 6] = _dup(w2[128:])
    s[:, 7] = _dup(SZ * wsum_b[128:])
    # gelu biases (Silu(1.702 y + 1.702 b) = 1.702 qgelu(y+b))
    s[:, 8] = 1.702 * mlp_in_b[0:128]
    s[:64, 9] = 1.702 * mlp_in_b[128:192]
    # diff biases (z-scaled); also the ppad t=0 planes
    s[:, 10] = SZ * off_fc1_b[64:192]
    s[:, 11] = _dup(SZ * off_fc1_b[0:64])
    # spconv taps: U = off ca 64:192, P = packed off ca 0:64
    for i, (dh, dw) in enumerate(TAPS):
        s[:, 12 + i] = wsp[64:192, dh + 1, dw + 1]
        s[:, 21 + i] = _dup(wsp[0:64, dh + 1, dw + 1])
    # negated conv_t-B edge weights for the full-span fix ops
    s[:, 30] = -_dup(w0[128:])
    s[:, 31] = -_dup(w2[128:])
    return s


def _aux_patterns():
    """Constant aux z-rows [4, NCOLS] bf16: (l>=1), (t==0), (t==15), ones."""
    out = np.zeros((4, NCOLS), np.float32)
    for k, (l0, nl) in enumerate(TILES):
        c0 = TILE_C0[k]
        for b in range(2):
            for l in range(nl):
                for t in range(T):
                    c = c0 + b * nl * T + l * T + t
                    out[0, c] = 0.0 if (nl == 15 and l == 0) else 1.0
                    out[1, c] = 1.0 if t == 0 else 0.0
                    out[2, c] = 1.0 if t == T - 1 else 0.0
                    out[3, c] = 1.0
    return out.astype(ml_dtypes.bfloat16)


def build_kernel():
    nc = bacc.Bacc("TRN2", target_bir_lowering=False, debug=False,
                   num_devices=NCORES)
    xt_d = nc.declare_dram_parameter("xt", [C, NCOLS], BF16, isOutput=False)
    x8_d = nc.declare_dram_parameter("x8", [128, 6, NCOLS], F8, isOutput=False)
    w1b_d = nc.declare_dram_parameter("w1b", [C, 192], BF16, isOutput=False)
    w18_d = nc.declare_dram_parameter("w18", [128, 6, 384], F8, isOutput=False)
    w2a_d = nc.declare_dram_parameter("w2a", [128, 2, C], F8, isOutput=False)
    w2b8_d = nc.declare_dram_parameter("w2b8", [128, C], F8, isOutput=False)
    w2g0_d = nc.declare_dram_parameter("w2g0", [128, C], BF16, isOutput=False)
    w2g1_d = nc.declare_dram_parameter("w2g1", [68, C], BF16, isOutput=False)
    aux_d = nc.declare_dram_parameter("auxz", [4, NCOLS], BF16, isOutput=False)
    sc_d = nc.declare_dram_parameter("scal", [128, NSCAL], F32, isOutput=False)
    out_d = nc.declare_dram_parameter("out", [C, NCOLS], BF16, isOutput=True)

    with TileContext(nc) as tc:
        with (
            tc.tile_pool(name="const", bufs=1) as cpool,
            tc.tile_pool(name="xin", bufs=3) as xpool,
            tc.tile_pool(name="tmp", bufs=2) as tpool,
            tc.tile_pool(name="grp", bufs=2) as gpool,
            tc.tile_pool(name="acc", bufs=2) as apool,
            tc.tile_pool(name="osbp", bufs=2) as opool_sb,
            tc.tile_pool(name="ypsum", bufs=1, space="PSUM") as ypool,
            tc.tile_pool(name="ypsum2", bufs=1, space="PSUM") as ypool2,
            tc.tile_pool(name="opsum", bufs=3, space="PSUM") as opool,
        ):
            # ---- input loads (xt on sync queue, x8 on scalar queue) ----
            xt_g = [None] * len(LOADG)
            x8_g = [None] * len(LOADG)

            def load_group(gi):
                g = LOADG[gi]
                c0 = TILE_C0[g[0]]
                gw = LOADW[gi]
                xt_g[gi] = xpool.tile([128, 6, MAXLW], BF16, name="xtg")
                x8_g[gi] = xpool.tile([128, 6, MAXLW], F8, name="x8g")
                src = xt_d[:, c0:c0 + gw].rearrange("(i p) c -> p i c", p=128)
                nc.sync.dma_start(out=xt_g[gi][:, :, 0:gw], in_=src)
                nc.sync.dma_start(out=x8_g[gi][:, :, 0:gw],
                                  in_=x8_d[:, :, c0:c0 + gw])

            load_group(0)

            # ---- weights + scalars (gpsimd queue; w2g0 first for warmup) ----
            w2g0 = cpool.tile([128, C], BF16, name="w2g0")
            nc.gpsimd.dma_start(out=w2g0[:], in_=w2g0_d[:])
            scal = cpool.tile([128, NSCAL], F32, name="scal")
            nc.gpsimd.dma_start(out=scal[:], in_=sc_d[:])

            # ---- PE warm-up: junk matmuls while x streams in ----
            for i in range(N_WARM):
                jt = opool.tile([128, 480], F32, name="ops")
                nc.tensor.matmul(jt[:, 0:448], w2g0[:, 0:128],
                                 w2g0[:, 0:448], start=True, stop=True)

            w18 = cpool.tile([128, 6, 384], F8, name="w18")
            nc.gpsimd.dma_start(out=w18[:], in_=w18_d[:])
            w1b = []
            for i in range(6):
                t = cpool.tile([128, 192], BF16, name=f"w1b_{i}")
                nc.gpsimd.dma_start(out=t[:],
                                    in_=w1b_d[i * 128:(i + 1) * 128, :])
                w1b.append(t)
            w2a = cpool.tile([128, 2, C], F8, name="w2a")
            nc.gpsimd.dma_start(out=w2a[:], in_=w2a_d[:])
            w2b8 = cpool.tile([128, C], F8, name="w2b8")
            nc.gpsimd.dma_start(out=w2b8[:], in_=w2b8_d[:])
            w2g1 = cpool.tile([68, C], BF16, name="w2g1")
            nc.gpsimd.dma_start(out=w2g1[:], in_=w2g1_d[:])

            # ---- full-width persistent z buffers ----
            zf01 = cpool.tile([128, 2, NCOLS], F8, name="zf01")
            zf2 = cpool.tile([128, NCOLS], F8, name="zf2")
            zg0 = cpool.tile([128, NCOLS], BF16, name="zg0")
            zg1 = cpool.tile([68, NCOLS], BF16, name="zg1")
            nc.sync.dma_start(out=zg1[64:68, :], in_=aux_d[:])

            # ---- padded diff grids (persistent; guards stay zero) ----
            ppadU = cpool.tile([128, PPADU_COLS], BF16, name="ppadU")
            ppadP = cpool.tile([128, PPADP_COLS], BF16, name="ppadP")
            nc.gpsimd.memset(ppadU[:], 0.0)
            nc.vector.memset(ppadP[:], 0.0)
            pu5 = ppadU[:, :].rearrange("p (b h w t) -> p b h w t",
                                        b=2, h=16, w=PADW)
            pp4 = ppadP[:, :].rearrange("p (h w t) -> p h w t", h=16, w=PADW)
            for b in (0, 1):
                t0u = pu5[:, b, 1:15, 1:15, 0:1]
                nc.scalar.activation(t0u, t0u, AF.Identity,
                                     bias=scal[:, 10:11], scale=0.0)
            t0p = pp4[:, 1:15, 1:15, 0:1]
            nc.scalar.activation(t0p, t0p, AF.Identity,
                                 bias=scal[:, 11:12], scale=0.0)

            load_group(1)
            load_group(2)

            def col(j, r0=0, r1=128):
                return scal[r0:r1, j:j + 1]

            # flat per-b views of the padded grids
            puf = ppadU[:, :].rearrange("p (b x) -> p b x", b=2)
            ppf = ppadP[:, :]
            puv = ppadU[:, :].rearrange("p (b l t) -> p b l t", b=2, t=T)
            ppv = ppadP[:, :].rearrange("p (l t) -> p l t", t=T)

            pb1_g = {}
            pb2_g = {}

            def x_views(k):
                for gi, g in enumerate(LOADG):
                    if k in g:
                        o = TILE_C0[k] - TILE_C0[g[0]]
                        w = TILES[k][1] * NL
                        xts = [xt_g[gi][:, i, o:o + w] for i in range(6)]
                        return xts, x8_g[gi][:, :, o:o + w]
                raise AssertionError

            def emit_A(k):
                nl = TILES[k][1]
                w = nl * NL
                xts, x8v = x_views(k)
                ys = []
                for m in range(3):
                    pool_m = ypool2 if m == 0 else ypool
                    yt = pool_m.tile([128, w], F32, name=f"y{m}")
                    for j in range(3):
                        nc.tensor.matmul(
                            yt[:, :], w18[:, 2 * j:2 * j + 2,
                                          m * 128:(m + 1) * 128],
                            x8v[:, 2 * j:2 * j + 2, :],
                            start=(j == 0), stop=(j == 2), perf_mode=PM)
                    ys.append(yt)
                y3 = ypool.tile([128, w], F32, name="y3")
                y4 = ypool.tile([64, w], F32, name="y4")
                for i in range(6):
                    nc.tensor.matmul(y3[:, :], w1b[i][:, 0:128], xts[i],
                                     start=(i == 0), stop=(i == 5))
                for i in range(6):
                    nc.tensor.matmul(y4[:, :], w1b[i][:, 128:192], xts[i],
                                     start=(i == 0), stop=(i == 5))
                return ys + [y3, y4]

            def emit_middle(k, ys, pb1, pb2, j):
                """j: tile's index within its tap group."""
                nl = TILES[k][1]
                w = nl * NL
                hw = nl * T
                c0 = TILE_C0[k]
                loff = 1 if k == 0 else 0
                y0, y1, y2, y3, y4 = ys

                # ---- gelu first: frees y3/y4 and unblocks B's zg deps ----
                nc.scalar.activation(zg0[:, c0:c0 + w], y3[:, :], AF.Silu,
                                     bias=col(8), scale=1.702)
                nc.scalar.activation(zg1[0:64, c0:c0 + w], y4[:, :], AF.Silu,
                                     bias=col(9, 0, 64), scale=1.702)

                # ---- conv_t chunk A (fc1 ca 0:128) -> zf01 ktile0 ----
                zaf = zf01[:, 0, c0:c0 + w]
                za = zaf.rearrange("p (b l t) -> p b l t", b=2, t=T)
                yv0 = y0[:, 0:w].rearrange("p (b l t) -> p b l t", b=2, t=T)
                nc.scalar.activation(zaf, y0[:, :], AF.Identity,
                                     bias=col(3), scale=col(1))
                for b in (0, 1):
                    nc.vector.scalar_tensor_tensor(
                        out=za[:, b, :, 1:], in0=yv0[:, b, :, :T - 1],
                        scalar=col(0), in1=za[:, b, :, 1:],
                        op0=OP.mult, op1=OP.add)
                for b in (0, 1):
                    nc.vector.scalar_tensor_tensor(
                        out=za[:, b, :, :T - 1], in0=yv0[:, b, :, 1:],
                        scalar=col(2), in1=za[:, b, :, :T - 1],
                        op0=OP.mult, op1=OP.add)

                # ---- evict y1, pack both halves (b -> partitions) ----
                tmp1 = tpool.tile([128, 480], BF16, name="tmp1")
                nc.scalar.activation(tmp1[:, :w], y1[:, :], AF.Copy)
                nc.gpsimd.dma_start(out=pb1[0:64, j, 0:hw],
                                    in_=tmp1[0:64, 0:hw])
                nc.gpsimd.dma_start(out=pb1[64:128, j, 0:hw],
                                    in_=tmp1[0:64, hw:2 * hw])
                nc.gpsimd.dma_start(out=pb2[0:64, j, 0:hw],
                                    in_=tmp1[64:128, 0:hw])
                nc.gpsimd.dma_start(out=pb2[64:128, j, 0:hw],
                                    in_=tmp1[64:128, hw:2 * hw])

                # ---- temporal diff -> padded grids (h-row k) ----
                lp0 = (k + 1) * PADW + 1
                tmp2 = tpool.tile([128, 480], BF16, name="tmp2")
                nc.scalar.activation(tmp2[:, :w], y2[:, :], AF.Copy)
                t2v = tmp2[:, 0:w].rearrange("p (b l t) -> p b l t",
                                             b=2, t=T)[:, :, loff:, :]
                pvU = puv[:, :, lp0:lp0 + HGRID, :]
                for b in (0, 1):
                    nc.vector.scalar_tensor_tensor(
                        out=pvU[:, b, :, 1:], in0=t2v[:, b, :, 1:],
                        scalar=col(10), in1=t2v[:, b, :, :T - 1],
                        op0=OP.add, op1=OP.subtract)
                pb2v = pb2[:, j, 0:hw].rearrange("p (l t) -> p l t", t=T)
                pvP = ppv[:, lp0:lp0 + HGRID, :]
                nc.vector.scalar_tensor_tensor(
                    out=pvP[:, :, 1:], in0=pb2v[:, loff:, 1:],
                    scalar=col(11), in1=pb2v[:, loff:, :T - 1],
                    op0=OP.add, op1=OP.subtract)

            taps_state = {}

            def emit_taps_phase(gidx, ph):
                """Phased tap emission so no single DVE batch starves the
                downstream queues. ph0: conv_t-B + qc unpacks; ph1: P taps
                0..7; ph2: P tap8 + qs unpacks + CLS masks; ph3: U taps."""
                tiles = TAPG[gidx]
                n = len(tiles)
                k0 = tiles[0]
                r0 = k0
                nl = TILES[k0][1]
                hw = nl * T
                loff = 1 if k0 == 0 else 0
                c0g = TILE_C0[k0]
                w = nl * NL
                pb1, pb2 = pb1_g[gidx], pb2_g[gidx]
                span = n * ROWE
                base = (r0 + 1) * ROWE

                if ph == 0:
                    # ---- conv_t chunk B on packed pb1 (group-batched).
                    # Full-span shifted taps cross (j,l) boundaries; the two
                    # strided fix ops subtract the wrong edge contributions.
                    qcB = gpool.tile([128, n, hw], BF16, name="qcB")
                    nc.scalar.activation(qcB[:, :, :], pb1[:, 0:n, 0:hw],
                                         AF.Identity, bias=col(7),
                                         scale=col(5))
                    m = n * hw
                    qf = qcB[:, :, :].rearrange("p j x -> p (j x)")
                    pf = pb1[:, 0:n, 0:hw].rearrange("p j x -> p (j x)")
                    nc.vector.scalar_tensor_tensor(
                        out=qf[:, 1:m], in0=pf[:, 0:m - 1],
                        scalar=col(4), in1=qf[:, 1:m],
                        op0=OP.mult, op1=OP.add)
                    nc.vector.scalar_tensor_tensor(
                        out=qf[:, 0:m - 1], in0=pf[:, 1:m],
                        scalar=col(6), in1=qf[:, 0:m - 1],
                        op0=OP.mult, op1=OP.add)
                    nlt = n * hw // T
                    qlt = qf.rearrange("p (l t) -> p l t", t=T)
                    plt = pf.rearrange("p (l t) -> p l t", t=T)
                    nc.vector.scalar_tensor_tensor(
                        out=qlt[:, 1:, 0:1], in0=plt[:, 0:nlt - 1, T - 1:T],
                        scalar=col(30), in1=qlt[:, 1:, 0:1],
                        op0=OP.mult, op1=OP.add)
                    nc.vector.scalar_tensor_tensor(
                        out=qlt[:, 0:nlt - 1, T - 1:T], in0=plt[:, 1:, 0:1],
                        scalar=col(31), in1=qlt[:, 0:nlt - 1, T - 1:T],
                        op0=OP.mult, op1=OP.add)
                    # unpack into zf01 ktile1 rows 0:64 (cast bf16->fp8)
                    zv = zf01[0:64, 1, c0g:c0g + n * w].rearrange(
                        "p (j b x) -> p j b x", j=n, b=2)
                    nc.gpsimd.dma_start(out=zv[:, :, 0, :], in_=qcB[0:64])
                    nc.gpsimd.dma_start(out=zv[:, :, 1, :], in_=qcB[64:128])

                elif ph == 1:
                    # ---- spconv chunk P taps 0..7 (packed) ----
                    accP = apool.tile([128, span], BF16, name="accP")
                    taps_state[(gidx, "accP")] = accP
                    for i, (dh, dw) in enumerate(TAPS[:8]):
                        off = base + (dh * PADW + dw) * T
                        src = ppf[:, off + T:off + span - T]
                        if i == 0:
                            nc.vector.tensor_scalar(
                                out=accP[:, T:span - T], in0=src,
                                scalar1=col(21), scalar2=None, op0=OP.mult)
                        else:
                            nc.vector.scalar_tensor_tensor(
                                out=accP[:, T:span - T], in0=src,
                                scalar=col(21 + i), in1=accP[:, T:span - T],
                                op0=OP.mult, op1=OP.add)

                elif ph == 2:
                    accP = taps_state.pop((gidx, "accP"))
                    nhw = (nl - loff) * T            # 224
                    qsB = gpool.tile([128, n * 224], BF16, name="qsB")
                    qs4 = qsB[:, 0:n * nhw].rearrange(
                        "p (j l t) -> p j l t", j=n, t=T)
                    qs2 = qsB[:, :].rearrange("p (j x) -> p j x", j=n)
                    inv2 = ppf[:, :].rearrange(
                        "p (r x) -> p r x", r=16)[:, r0 + 2:r0 + 2 + n,
                                                  2 * T:2 * T + 224]
                    acv2 = accP[:, :].rearrange(
                        "p (j x) -> p j x", j=n)[:, :, T:T + 224]
                    nc.vector.scalar_tensor_tensor(
                        out=qs2, in0=inv2, scalar=col(29), in1=acv2,
                        op0=OP.mult, op1=OP.add)
                    # unpack into zf01 ktile1 rows 64:128 (cast to fp8)
                    zu = zf01[64:128, 1, c0g:c0g + n * w].rearrange(
                        "p (j b l t) -> p j b l t", j=n, b=2, t=T)
                    nc.gpsimd.dma_start(out=zu[:, :, 0, loff:, :],
                                        in_=qs4[0:64])
                    nc.gpsimd.dma_start(out=zu[:, :, 1, loff:, :],
                                        in_=qs4[64:128])
                    if k0 == 0:
                        # zero the CLS cols of the offset-branch z rows
                        zk1v = zf01[64:128, 1, c0g:c0g + w].rearrange(
                            "p (b l t) -> p b l t", b=2, t=T)
                        nc.vector.memset(zk1v[:, :, 0:1, :], 0.0)
                        zf2v = zf2[:, c0g:c0g + w].rearrange(
                            "p (b l t) -> p b l t", b=2, t=T)
                        nc.vector.memset(zf2v[:, :, 0:1, :], 0.0)

                else:
                    # ---- spconv chunk U: taps 0..7 + tap 8 -> zf2 ----
                    accU = apool.tile([128, 2, span], BF16, name="accU")
                    tmpU = apool.tile([128, 2, span], BF16, name="tmpU")
                    for i, (dh, dw) in enumerate(TAPS[:8]):
                        off = base + (dh * PADW + dw) * T
                        src = puf[:, :, off + T:off + span - T]
                        if i == 0:
                            nc.vector.tensor_scalar(
                                out=accU[:, :, T:span - T], in0=src,
                                scalar1=col(12), scalar2=None, op0=OP.mult)
                        else:
                            # ts at 4x + TT-add at 2x beats stt's 1x rate
                            nc.vector.tensor_scalar(
                                out=tmpU[:, :, T:span - T], in0=src,
                                scalar1=col(12 + i), scalar2=None,
                                op0=OP.mult)
                            nc.vector.tensor_tensor(
                                out=accU[:, :, T:span - T],
                                in0=accU[:, :, T:span - T],
                                in1=tmpU[:, :, T:span - T], op=OP.add)
                    # tap 8: shifted window stays contiguous per (b,row)
                    zrows = zf2[:, c0g:c0g + n * w].rearrange(
                        "p (j x) -> p j x", j=n)
                    for b in (0, 1):
                        inv = puf[:, b, :].rearrange(
                            "p (r x) -> p r x", r=16)[:, r0 + 2:r0 + 2 + n,
                                                      2 * T:2 * T + 224]
                        acv = accU[:, b, :].rearrange(
                            "p (j x) -> p j x", j=n)[:, :, T:T + 224]
                        o0 = b * hw + loff * T
                        nc.vector.scalar_tensor_tensor(
                            out=zrows[:, :, o0:o0 + 224], in0=inv,
                            scalar=col(20), in1=acv, op0=OP.mult, op1=OP.add)

            def emit_B_tile(k):
                nl = TILES[k][1]
                w = nl * NL
                c0 = TILE_C0[k]
                osb = opool_sb.tile([128, 6, 480], BF16, name="osb")
                for m in range(6):
                    m0 = m * 128
                    ot = opool.tile([128, 480], F32, name="ops")
                    nc.tensor.matmul(ot[:, 0:w], w2a[:, :, m0:m0 + 128],
                                     zf01[:, :, c0:c0 + w], start=True,
                                     stop=False, perf_mode=PM)
                    nc.tensor.matmul(ot[:, 0:w], w2b8[:, m0:m0 + 128],
                                     zf2[:, c0:c0 + w], start=False,
                                     stop=False)
                    nc.tensor.matmul(ot[:, 0:w], w2g0[:, m0:m0 + 128],
                                     zg0[:, c0:c0 + w], start=False,
                                     stop=False)
                    nc.tensor.matmul(ot[:, 0:w], w2g1[:, m0:m0 + 128],
                                     zg1[0:68, c0:c0 + w], start=False,
                                     stop=True)
                    nc.scalar.activation(osb[:, m, 0:w], ot[:, 0:w],
                                         AF.Copy, scale=1.0 / G)
                dst = out_d[:, c0:c0 + w].rearrange("(m p) c -> p m c", p=128)
                nc.sync.dma_start(out=dst, in_=osb[:, :, 0:w])

            # ---- main pipeline ----
            for gi, tiles in enumerate(TAPG):
                n = len(tiles)
                hw_g = TILES[tiles[0]][1] * T
                pb1_g[gi] = gpool.tile([128, n, hw_g], BF16, name="pb1")
                pb2_g[gi] = gpool.tile([128, n, hw_g], BF16, name="pb2")
                prev2 = TAPG[gi - 2] if gi >= 2 else []
                ph = 0
                for idx, k in enumerate(tiles):
                    if idx < len(prev2):
                        emit_B_tile(prev2[idx])
                    ys = emit_A(k)
                    if k in LOAD_TRIGGER:
                        load_group(LOAD_TRIGGER[k])
                    emit_middle(k, ys, pb1_g[gi], pb2_g[gi], idx)
                    if gi >= 1 and ph < 4:
                        emit_taps_phase(gi - 1, ph)
                        ph += 1
                for idx in range(len(tiles), len(prev2)):
                    emit_B_tile(prev2[idx])
                while gi >= 1 and ph < 4:
                    emit_taps_phase(gi - 1, ph)
                    ph += 1
            # tail: interleave the last tap phases with B of the
            # second-to-last group so the PE stays fed
            last = len(TAPG) - 1
            emit_B_tile(TAPG[-2][0])
            emit_taps_phase(last, 0)
            emit_taps_phase(last, 1)
            emit_B_tile(TAPG[-2][1])
            emit_taps_phase(last, 2)
            emit_taps_phase(last, 3)
            for k in TAPG[-1]:
                emit_B_tile(k)

    nc.compile()
    return nc


_cached = {}


def _get_kernel():
    if "nc" not in _cached:
        _cached["nc"] = build_kernel()
    return _cached["nc"]


def _host_xt(x):
    """x (L, 256, C) f32 -> (8, C, NCOLS) f32, per-tile b-major token order."""
    out = np.empty((NCORES, C, NCOLS), np.float32)
    x5 = x.reshape(L, NCORES, 2, T, C)
    for k, (l0, nl) in enumerate(TILES):
        blk = x5[l0:l0 + nl]                      # (nl, 8, 2, T, C)
        blk = blk.transpose(1, 4, 2, 0, 3)        # (8, C, 2, nl, T)
        out[:, :, TILE_C0[k]:TILE_C0[k + 1]] = blk.reshape(NCORES, C, nl * NL)
    return out


def _host_out(outT):
    """outT (8, C, NCOLS) -> out (L, 256, C) f32."""
    out = np.empty((L, NCORES * NL, C), np.float32)
    for k, (l0, nl) in enumerate(TILES):
        blk = outT[:, :, TILE_C0[k]:TILE_C0[k + 1]].astype(np.float32)
        blk = blk.reshape(NCORES, C, 2, nl, T)    # (8, C, 2, nl, T)
        blk = blk.transpose(3, 0, 2, 4, 1)        # (nl, 8, 2, T, C)
        out[l0:l0 + nl] = blk.reshape(nl, NCORES * NL, C)
    return out


def kernel(x, T, fc1_w, fc1_b, conv_w, conv_b, fc2_w, fc2_b,
           off_fc1_w, off_fc1_b, off_conv_w, off_conv_b, off_fc2_w, off_fc2_b,
           mlp_in_w, mlp_in_b, mlp_out_w, mlp_out_b):
    bf = ml_dtypes.bfloat16
    f8 = ml_dtypes.float8_e4m3fn
    x = np.asarray(x, np.float32)
    to_np = lambda a: np.asarray(a, np.float32)
    (fc1_w, fc1_b, conv_w, conv_b, fc2_w, fc2_b, off_fc1_w, off_fc1_b,
     off_conv_w, off_conv_b, off_fc2_w, off_fc2_b, mlp_in_w, mlp_in_b,
     mlp_out_w, mlp_out_b) = map(to_np, (
        fc1_w, fc1_b, conv_w, conv_b, fc2_w, fc2_b, off_fc1_w, off_fc1_b,
        off_conv_w, off_conv_b, off_fc2_w, off_fc2_b, mlp_in_w, mlp_in_b,
        mlp_out_w, mlp_out_b))

    xtf = _host_xt(x)                       # (8, C, NCOLS) f32
    xt = xtf.astype(bf)
    x8 = np.ascontiguousarray(
        xtf.astype(f8).reshape(NCORES, 6, 128, NCOLS).transpose(0, 2, 1, 3))

    w1cat = np.concatenate([fc1_w, off_fc1_w], axis=1)      # (768, 384)
    w18 = np.ascontiguousarray(
        (SW * w1cat).astype(f8).reshape(6, 128, 384).transpose(1, 0, 2))
    w1b = mlp_in_w.astype(bf)                               # (768, 192)

    w2kt0 = SW * fc2_w[0:128]
    w2kt1 = np.concatenate([SW * fc2_w[128:192], SW * off_fc2_w[0:64]], 0)
    w2a = np.ascontiguousarray(
        np.stack([w2kt0, w2kt1], 0).astype(f8).transpose(1, 0, 2))
    w2b8 = (SW * off_fc2_w[64:192]).astype(f8)

    w2g0 = (G / 1.702 * mlp_out_w[0:128]).astype(bf)
    w0, w1c, w2c = conv_w[:, 0, 0], conv_w[:, 0, 1], conv_w[:, 0, 2]
    aux_w2 = np.stack([
        G * (off_conv_b @ off_fc2_w + off_fc2_b),
        G * ((-w0 * fc1_b) @ fc2_w),
        G * ((-w2c * fc1_b) @ fc2_w),
        G * (fc2_b + mlp_out_b),
    ], 0)
    w2g1 = np.concatenate([G / 1.702 * mlp_out_w[128:192], aux_w2],
                          0).astype(bf)

    scal = _pack_scalars(conv_w, conv_b, fc1_b, mlp_in_b, off_fc1_b,
                         off_conv_w)
    auxz = _aux_patterns()

    nc = _get_kernel()
    in_maps = [{"xt": xt[i], "x8": x8[i], "w1b": w1b, "w18": w18,
                "w2a": w2a, "w2b8": w2b8, "w2g0": w2g0, "w2g1": w2g1,
                "auxz": auxz, "scal": scal}
               for i in range(NCORES)]
    res = bass_utils.run_bass_kernel_spmd(nc, in_maps,
                                          core_ids=list(range(NCORES)))
    _cached["last_result"] = res

    outT = np.stack([np.asarray(res.results[i]["out"]) for i in range(NCORES)])
    return np.ascontiguousarray(_host_out(outT))
